# revision 52
# baseline (speedup 1.0000x reference)
"""DeepTreeLSTM Trainium2 Bass kernel (v2).

B=256 perfect binary trees (511 nodes, BFS layout), ChildSum TreeLSTM
bottom-up + MLP head. Data-parallel over trees: 32 trees per NeuronCore
x 8 cores. Device tensors use a transposed feature-on-partition layout
[H (2 chunks of 128 partitions), columns], columns tree-major.

Within each tree, every level is stored in BIT-REVERSED node order, so
the two children of any parent sit in opposite halves of the tree's
column range at the same offset. All sibling pair-sums (h_tild, c_agg)
then become contiguous half+half tensor adds (full DVE/GPSIMD rate
instead of stride-2), and each level's outputs land already in the
bit-reversed order its parent level expects. The leaf permutation is
applied to X on the host for free. rev(255)=255 keeps the "last leaf"
(head's excluded node) in the last column; the root is unaffected.

Engine layout per level block (512 parent cols):
  PE:    f = U_f @ ch_h (4 matmuls/1024 children), iou = U_iou @ h_tild
         (12 matmuls), all N=512 into a rotating 4x[P,2,512] PSUM pool.
  ACT:   f sigmoid per chunk over [P,2,512] (bias=U_f_b chunk), iou
         gates as 3x [P,2,512] instrs (sig i, sig o, tanh u), tanh(c).
  DVE:   h_tild half+half add, per-tree h_tild reduce (hsum), f*c,
         i*u, c=iu+c_agg, h=o*tanh(c).
  GPSIMD: c_agg half+half add.
The tanh(c)/h tail of block j is emitted after block j+1's gates so the
ACT stream never waits on the DVE chain (software pipelining). Deep
levels (d<=4) run in two independent 16-tree halves so consecutive
levels overlap.

Contract notes vs the reference: the h input is unused (shape only);
c, b_iou, b_in, b_mid, b_out are all-zero per the problem's input spec,
so the kernel drops them (only U_f_b is a live bias).
"""

import os
import sys

import ml_dtypes
import numpy as np

BFNP = ml_dtypes.bfloat16
F8NP = ml_dtypes.float8_e4m3

for _p in ("/opt/trn_rl_repo", "/root/.axon_site/_ro/trn_rl_repo"):
    if os.path.isdir(_p) and _p not in sys.path:
        sys.path.insert(0, _p)

import concourse.bass as bass
import concourse.mybir as mybir
import concourse.tile as tile
from concourse import bacc
from concourse.bass_utils import run_bass_kernel_spmd

P = 128
F32 = mybir.dt.float32
BF16 = mybir.dt.bfloat16
F8 = mybir.dt.float8e4
H = 256           # hidden size (2 partition chunks)
NB = 32           # trees per core
LEAF = 256        # leaves per tree
COLS = NB * LEAF  # leaf columns per core = 8192
BLK = 512
AF = mybir.ActivationFunctionType
OP = mybir.AluOpType

_PROG = None


def _build_program():
    nc = bacc.Bacc("TRN2", target_bir_lowering=False, debug=False,
                   num_devices=8)

    xT = nc.dram_tensor("xT", [P, 2, COLS], BF16, kind="ExternalInput")
    wiouT = nc.dram_tensor("wiouT", [P, 2, 768], BF16, kind="ExternalInput")
    uiouT = nc.dram_tensor("uiouT", [P, 2, 768], BF16, kind="ExternalInput")
    ufT = nc.dram_tensor("ufT", [P, 2, 256], BF16, kind="ExternalInput")
    ufb = nc.dram_tensor("ufb", [P, 2], F32, kind="ExternalInput")
    winT = nc.dram_tensor("winT", [P, 5, P], BF16, kind="ExternalInput")
    emoT = nc.dram_tensor("emoT", [P, NB], BF16, kind="ExternalInput")
    wmidT = nc.dram_tensor("wmidT", [P, 64], F32, kind="ExternalInput")
    woutT = nc.dram_tensor("woutT", [P, 4], F32, kind="ExternalInput")
    out_t = nc.dram_tensor("out_t", [4, NB], F32, kind="ExternalOutput")

    with tile.TileContext(nc) as tc:
        with (
            tc.tile_pool(name="wp", bufs=1) as wp,
            tc.tile_pool(name="pers", bufs=1) as pers,
        ):
            wiou_sb = wp.tile([P, 2, 768], BF16)
            uiou_sb = wp.tile([P, 2, 768], BF16)
            uf_sb = wp.tile([P, 2, 256], BF16)
            ufb_sb = wp.tile([P, 2], F32)
            win_sb = wp.tile([P, 5, P], BF16)
            emo_sb = wp.tile([P, NB], BF16)
            wmid_sb = wp.tile([P, 64], F32)
            wout_sb = wp.tile([P, 4], F32)
            # wiou is the first weight the matmul stream needs; issue it
            # via the GPSIMD queue (live ~2us before the sync path at
            # kernel start) and split it across two queues
            nc.gpsimd.dma_start(wiou_sb[:, 0, :], wiouT[:, 0, :])
            nc.gpsimd.dma_start(wiou_sb[:, 1, :], wiouT[:, 1, :])
            for sb, dr in ((uiou_sb, uiouT), (uf_sb, ufT),
                           (ufb_sb, ufb), (win_sb, winT), (emo_sb, emoT),
                           (wmid_sb, wmidT), (wout_sb, woutT)):
                nc.sync.dma_start(sb[:], dr[:])

            # per-level h/c tensors, bit-reversed node order within trees
            hL = {8: pers.tile([P, 2, COLS], BF16, name="h8")}
            cL = {8: pers.tile([P, 2, COLS], BF16, name="c8")}
            for d in range(7, -1, -1):
                m = NB * (2 ** d)
                hL[d] = pers.tile([P, 2, m], BF16, name=f"h{d}")
                cL[d] = pers.tile([P, 2, m], BF16, name=f"c{d}")
            hsum = pers.tile([P, 2, NB], F32)
            hlast = pers.tile([P, 2, NB], F32)
            nc.vector.memset(hsum[:], 0.0)

            with (
                tc.tile_pool(name="pps", bufs=4, space="PSUM") as pps,
                tc.tile_pool(name="wk", bufs=1) as wk,
            ):
                # ---- ramp priming: load the sigmoid/tanh ACT table while
                # the input DMAs are still in flight ----
                scr = wk.tile([P, 16], BF16, name="scr")
                scr2 = wk.tile([P, 16], F32, name="scr2")
                nc.vector.memset(scr[:], 0.0)
                nc.scalar.activation(scr2[:, :8], scr[:, :8], AF.Sigmoid)

                def iou_mm_gates(rhs, w_sb, n, tag, fine=False, dr=False,
                                 scale=1.0):
                    """iou = W @ rhs, then 3 ACT instrs (sig i, sig o,
                    tanh u) -> bf16 SBUF. dr=True uses fp8 DoubleRow
                    matmuls (K=256 in one op; weights pre-scaled x16 on
                    the host, descaled via the free ACT scale). fine=True
                    drains each chunk separately so the pipeline fills
                    faster at kernel start."""
                    io_sb = wk.tile([P, 4, BLK], BF16, tag="iob", bufs=3,
                                    name=f"io_{tag}")
                    u_sb = wk.tile([P, 2, BLK], BF16, tag="ub", bufs=3,
                                   name=f"u_{tag}")
                    for g in range(3):
                        pg = pps.tile([P, 2, BLK], F32, tag="psA",
                                      name=f"pg_{tag}_{g}")
                        func = AF.Sigmoid if g < 2 else AF.Tanh
                        for ch in range(2):
                            mm = g * 2 + ch
                            if dr:
                                nc.tensor.matmul(
                                    pg[:, ch, :n],
                                    w_sb[:, :, mm * P:(mm + 1) * P],
                                    rhs, start=True, stop=True,
                                    perf_mode=mybir.MatmulPerfMode.DoubleRow)
                            else:
                                for k in range(2):
                                    nc.tensor.matmul(
                                        pg[:, ch, :n],
                                        w_sb[:, k, mm * P:(mm + 1) * P],
                                        rhs[:, k, :],
                                        start=(k == 0), stop=(k == 1))
                            if fine:
                                dst = io_sb[:, 2 * g + ch, :n] if g < 2 \
                                    else u_sb[:, ch, :n]
                                nc.scalar.activation(dst, pg[:, ch, :n], func,
                                                     scale=scale)
                        if not fine:
                            dst = io_sb[:, 2 * g:2 * g + 2, :n] if g < 2 \
                                else u_sb[:, :, :n]
                            nc.scalar.activation(dst, pg[:, :, :n], func,
                                                 scale=scale)
                    return io_sb, u_sb

                # ---------------- leaf phase ----------------
                # software-pipelined tail: tanh(c)/h of block j-1 emitted
                # after block j's gates
                leaf_tail = []

                def leaf_head(b):
                    s = slice(b * BLK, (b + 1) * BLK)
                    xk = wk.tile([P, 2, BLK], BF16, tag="xk", bufs=3,
                                 name=f"xk_{b}")
                    if b == 0:
                        # first block: early GPSIMD-issued DMA, halves on
                        # two queues so the pipeline fills sooner
                        nc.gpsimd.dma_start(xk[:, 0, :], xT[:, 0, s])
                        nc.gpsimd.dma_start(xk[:, 1, :], xT[:, 1, s])
                    else:
                        nc.sync.dma_start(xk[:], xT[:, :, s])
                    io_sb, u_sb = iou_mm_gates(xk[:], wiou_sb, BLK, f"L{b}",
                                               fine=(b == 0))
                    # c = i*u (initial c is zero at leaves)
                    nc.vector.tensor_mul(cL[8][:, :, s], io_sb[:, 0:2, :],
                                         u_sb[:])
                    return io_sb

                def leaf_tail_fn(b0, io0, b1, io1):
                    # one tanh over both blocks' c, then per-block h muls
                    s2 = slice(b0 * BLK, (b1 + 1) * BLK)
                    t_sb = wk.tile([P, 2, 2 * BLK], BF16, tag="tb", bufs=2,
                                   name=f"tl_{b0}")
                    nc.scalar.activation(t_sb[:], cL[8][:, :, s2], AF.Tanh)
                    for b, io_sb in ((b0, io0), (b1, io1)):
                        s = slice(b * BLK, (b + 1) * BLK)
                        o = b - b0
                        nc.vector.tensor_mul(
                            hL[8][:, :, s], io_sb[:, 2:4, :],
                            t_sb[:, :, o * BLK:(o + 1) * BLK])

                ios = {}
                nb = COLS // BLK
                for b in range(nb):
                    ios[b] = leaf_head(b)
                    # pair (b-2, b-1) tails one block behind the gates
                    if b >= 2 and b % 2 == 0:
                        leaf_tail_fn(b - 2, ios.pop(b - 2),
                                     b - 1, ios.pop(b - 1))
                leaf_tail_fn(nb - 2, ios.pop(nb - 2), nb - 1, ios.pop(nb - 1))
                # last leaf of each tree sits at per-tree col 255 (rev==id)
                nc.vector.tensor_copy(hlast[:],
                                      hL[8][:, :, LEAF - 1::LEAF])

                # ---------------- internal levels ----------------
                pair_ctr = [0]

                def level_unit(d, t0, t1, tag, defer=False, poly=False):
                    """Process level d for trees [t0, t1): produces
                    hL[d]/cL[d] cols [t0*2^d, t1*2^d) from level d+1."""
                    m_t = 2 ** d          # parents per tree
                    ch_h, ch_c = hL[d + 1], cL[d + 1]
                    # views splitting each tree's children into halves
                    chv_h = ch_h.rearrange("p k (t two n) -> p k t two n",
                                           two=2, n=m_t)
                    chv_c = ch_c.rearrange("p k (t two n) -> p k t two n",
                                           two=2, n=m_t)
                    p0 = t0 * m_t
                    pcols = (t1 - t0) * m_t
                    nblk = (pcols + BLK - 1) // BLK
                    tpb = max(1, BLK // m_t)       # trees per parent block

                    pend = []

                    def blk_tail(pairs):
                        # tanh over the pairs' c span (ACT, or DVE poly on
                        # the big levels where |c|<0.9), then h muls
                        j0, w0 = pairs[0][0], pairs[0][1]
                        jl, wl = pairs[-1][0], pairs[-1][1]
                        s2 = slice(p0 + j0 * BLK, p0 + jl * BLK + wl)
                        tw = jl * BLK + wl - j0 * BLK
                        t_sb = wk.tile([P, 2, 2 * BLK], BF16, tag="tb",
                                       bufs=2, name=f"t_{tag}_{j0}")
                        cseg = cL[d][:, :, s2]
                        use_poly = False
                        pair_ctr[0] += 1
                        if use_poly:
                            s_t = wk.tile([P, 2, 2 * BLK], BF16, tag="ps1",
                                          bufs=1, name=f"s_{tag}_{j0}")
                            q_t = wk.tile([P, 2, 2 * BLK], BF16, tag="ps2",
                                          bufs=1, name=f"q_{tag}_{j0}")
                            nc.vector.tensor_mul(s_t[:, :, :tw], cseg, cseg)
                            nc.vector.tensor_scalar(q_t[:, :, :tw],
                                                    s_t[:, :, :tw], TA5, TA3,
                                                    OP.mult, OP.add)
                            nc.vector.tensor_mul(s_t[:, :, :tw],
                                                 s_t[:, :, :tw],
                                                 q_t[:, :, :tw])
                            nc.vector.tensor_scalar_add(q_t[:, :, :tw],
                                                        s_t[:, :, :tw], TA1)
                            nc.vector.tensor_mul(t_sb[:, :, :tw], cseg,
                                                 q_t[:, :, :tw])
                        else:
                            nc.scalar.activation(t_sb[:, :, :tw], cseg,
                                                 AF.Tanh)
                        for j, w, io_sb in pairs:
                            s = slice(p0 + j * BLK, p0 + j * BLK + w)
                            o = (j - j0) * BLK
                            nc.vector.tensor_mul(hL[d][:, :, s],
                                                 io_sb[:, 2:4, :w],
                                                 t_sb[:, :, o:o + w])

                    for j in range(nblk):
                        w = min(BLK, pcols - j * BLK)
                        s = slice(p0 + j * BLK, p0 + j * BLK + w)
                        ta = t0 + j * tpb
                        tb_ = min(t1, ta + tpb)
                        nt = tb_ - ta

                        # h_tild: contiguous half+half add (DVE — it feeds
                        # the iou matmuls, keep it off the slow GPSIMD)
                        ht = wk.tile([P, 2, BLK], BF16, tag="ht", bufs=3,
                                     name=f"ht_{tag}_{j}")
                        htv = ht.rearrange("p k (t n) -> p k t n", n=m_t)
                        nc.vector.tensor_add(htv[:, :, :nt, :],
                                             chv_h[:, :, ta:tb_, 0, :],
                                             chv_h[:, :, ta:tb_, 1, :])

                        # f gates + f*c over this block's 2w children,
                        # PSUM per chunk so the sigmoid bias stays scalar
                        c0 = 2 * (p0 + j * BLK)
                        cw = 2 * w
                        nh = (cw + BLK - 1) // BLK   # 512-wide halves (1or2)
                        f_sb = wk.tile([P, 2, 2, BLK], BF16, tag="fb",
                                       bufs=3, name=f"f_{tag}_{j}")
                        for g in range(2):
                            pf = pps.tile([P, 2, BLK], F32, tag="psA",
                                          name=f"pf_{tag}_{j}_{g}")
                            for k in range(2):
                                for hh in range(nh):
                                    hw = min(BLK, cw - hh * BLK)
                                    cs = slice(c0 + hh * BLK,
                                               c0 + hh * BLK + hw)
                                    nc.tensor.matmul(
                                        pf[:, hh, :hw],
                                        uf_sb[:, k, g * P:(g + 1) * P],
                                        ch_h[:, k, cs],
                                        start=(k == 0), stop=(k == 1))
                            hw = min(BLK, cw - (nh - 1) * BLK)
                            nc.scalar.activation(
                                f_sb[:, g, :nh, :hw] if nh > 1 or hw == BLK
                                else f_sb[:, g, 0, :hw],
                                pf[:, :nh, :hw] if nh > 1 or hw == BLK
                                else pf[:, 0, :hw],
                                AF.Sigmoid, bias=ufb_sb[:, g:g + 1])
                        # f*c in place over children c
                        for hh in range(nh):
                            hw = min(BLK, cw - hh * BLK)
                            cs = slice(c0 + hh * BLK, c0 + hh * BLK + hw)
                            nc.vector.tensor_mul(ch_c[:, :, cs],
                                                 f_sb[:, :, hh, :hw],
                                                 ch_c[:, :, cs])
                        # c_agg: contiguous half+half add (GPSIMD)
                        cav = cL[d].rearrange("p k (t n) -> p k t n", n=m_t)
                        nc.gpsimd.tensor_add(cav[:, :, ta:tb_, :],
                                             chv_c[:, :, ta:tb_, 0, :],
                                             chv_c[:, :, ta:tb_, 1, :])

                        # iou from h_tild
                        io_sb, u_sb = iou_mm_gates(ht[:, :, :w], uiou_sb, w,
                                                   f"{tag}_{j}")
                        # c = i*u + c_agg
                        iu = wk.tile([P, 2, BLK], BF16, tag="iu", bufs=3,
                                     name=f"iu_{tag}_{j}")
                        nc.vector.tensor_mul(iu[:, :, :w], io_sb[:, 0:2, :w],
                                             u_sb[:, :, :w])
                        nc.vector.tensor_add(cL[d][:, :, s], iu[:, :, :w],
                                             cL[d][:, :, s])
                        # per-tree reduce of h_tild = per-tree colsum of
                        # level d+1 h (feeds the head's inner mean); emitted
                        # after the c chain to keep it off the critical path
                        part = wk.tile([P, 2, NB], F32, tag="part", bufs=2,
                                       name=f"part_{tag}_{j}")
                        nc.vector.tensor_reduce(
                            part[:, :, :nt], htv[:, :, :nt, :],
                            axis=mybir.AxisListType.X, op=OP.add)
                        nc.vector.tensor_add(hsum[:, :, ta:tb_],
                                             part[:, :, :nt],
                                             hsum[:, :, ta:tb_])
                        pend.append((j, w, io_sb))
                        # pair (j-2, j-1) tails one block behind the gates
                        if j >= 2 and j % 2 == 0:
                            blk_tail([pend.pop(0), pend.pop(0)])
                    if defer:
                        return lambda: blk_tail(pend)
                    blk_tail(pend)

                for d in range(7, 4, -1):
                    level_unit(d, 0, NB, f"B{d}", poly=True)
                # deep levels: two independent 16-tree halves per level,
                # with each unit's tanh/h tail deferred past the next
                # unit's gates so consecutive units overlap on the ACT
                prev_tail = None
                for d in range(4, -1, -1):
                    for hf, (ta, tb_) in enumerate(((0, NB // 2),
                                                    (NB // 2, NB))):
                        t = level_unit(d, ta, tb_, f"B{d}{'ab'[hf]}",
                                       defer=True)
                        if prev_tail is not None:
                            prev_tail()
                        prev_tail = t
                prev_tail()

                # ---------------- head (fp32 tail) ----------------
                inner = wk.tile([P, 2, NB], BF16, name="inner")
                nc.vector.tensor_sub(inner[:], hsum[:], hlast[:])
                nc.vector.tensor_scalar_mul(inner[:], inner[:], 1.0 / 509.0)
                y2_sb = wk.tile([P, NB], F32, name="y2")
                nc.vector.memset(y2_sb[:], 0.0)

                # root-h contributions last so the inner/emo matmuls can
                # run before the final level finishes
                h_root = hL[0]
                py1 = pps.tile([P, NB], F32, tag="psA", name="py1")
                chunks = [(2, inner[:, 0, :]), (3, inner[:, 1, :]),
                          (4, emo_sb[:]), (0, h_root[:, 0, :]),
                          (1, h_root[:, 1, :])]
                for i, (k, rhs) in enumerate(chunks):
                    nc.tensor.matmul(py1[:], win_sb[:, k, :], rhs,
                                     start=(i == 0), stop=(i == 4))
                y1_sb = wk.tile([P, NB], F32, name="y1")
                nc.scalar.activation(y1_sb[:], py1[:], AF.Relu)
                py2 = pps.tile([64, NB], F32, tag="psA", name="py2")
                nc.tensor.matmul(py2[:], wmid_sb[:], y1_sb[:])
                nc.scalar.activation(y2_sb[:64, :], py2[:], AF.Relu)
                po = pps.tile([4, NB], F32, tag="psA", name="po")
                nc.tensor.matmul(po[:], wout_sb[:], y2_sb[:])
                o_sb = wk.tile([4, NB], F32, name="osb")
                nc.scalar.activation(o_sb[:], po[:], AF.Sigmoid)
                nc.sync.dma_start(out_t[:], o_sb[:])

    nc.finalize()
    return nc


def _bitrev(n_bits):
    n = 1 << n_bits
    r = np.zeros(n, np.int64)
    for i in range(n):
        b = 0
        for j in range(n_bits):
            if i & (1 << j):
                b |= 1 << (n_bits - 1 - j)
        r[i] = b
    return r


_PERM = _bitrev(8)  # leaf j -> storage position


def _chunked(w):
    """[K, M] host array -> [P, K//P, M] device layout (K on partitions)."""
    k, m = w.shape
    return np.ascontiguousarray(w.reshape(k // P, P, m).transpose(1, 0, 2))


def _prep_shared(W_iou, U_iou, b_iou, U_f_w, U_f_b, W_in, b_in, W_mid, b_mid,
                 W_out, b_out):
    f = np.float32
    wiouT = _chunked(np.ascontiguousarray(W_iou.T).astype(f)).astype(BFNP)
    uiouT = _chunked(np.ascontiguousarray(U_iou.T).astype(f)).astype(BFNP)
    ufT = _chunked(np.ascontiguousarray(U_f_w.T).astype(f)).astype(BFNP)
    ufb_h = np.ascontiguousarray(U_f_b.reshape(2, P).T).astype(f)
    winT = np.zeros((640, P), f)
    winT[:544] = W_in.T
    winT = _chunked(winT).astype(BFNP)
    wmidT = np.ascontiguousarray(W_mid.T).astype(f)
    woutT = np.zeros((P, 4), f)
    woutT[:64] = W_out.T
    return dict(wiouT=wiouT, uiouT=uiouT, ufT=ufT, ufb=ufb_h,
                winT=winT, wmidT=wmidT, woutT=woutT)


def _run(X, emo, shared, trace=False):
    global _PROG
    if _PROG is None:
        _PROG = _build_program()
    nc = _PROG

    inv = np.argsort(_PERM)  # storage position -> leaf (gather index)
    in_maps = []
    for cc in range(8):
        Xc = X[cc * NB:(cc + 1) * NB, 255:511, :][:, inv, :]
        xT = Xc.transpose(2, 0, 1).reshape(256, COLS)
        xT = np.ascontiguousarray(
            xT.reshape(2, P, COLS).transpose(1, 0, 2)).astype(BFNP)
        emoT = np.zeros((P, NB), BFNP)
        emoT[:32] = emo[cc * NB:(cc + 1) * NB].T.astype(BFNP)
        in_maps.append(dict(xT=xT, emoT=emoT, **shared))

    res = None
    for attempt in range(3):
        try:
            res = run_bass_kernel_spmd(nc, in_maps, core_ids=list(range(8)),
                                       trace=trace)
            break
        except Exception:
            if attempt == 2:
                raise
    out = np.concatenate([res.results[cc]["out_t"].T for cc in range(8)],
                         axis=0)
    return np.ascontiguousarray(out.astype(np.float32)), res


def kernel(X, h, c, emo, W_iou, U_iou, b_iou, U_f_w, U_f_b,
           W_in, b_in, W_mid, b_mid, W_out, b_out, **kwargs):
    X = np.asarray(X, np.float32)
    emo = np.asarray(emo, np.float32)
    shared = _prep_shared(np.asarray(W_iou), np.asarray(U_iou),
                          np.asarray(b_iou), np.asarray(U_f_w),
                          np.asarray(U_f_b), np.asarray(W_in),
                          np.asarray(b_in), np.asarray(W_mid),
                          np.asarray(b_mid), np.asarray(W_out),
                          np.asarray(b_out))
    out, _ = _run(X, emo, shared)
    return out
